# revision 1
# baseline (speedup 1.0000x reference)
"""Trainium2 Bass kernel for nn_GruAgent (GRU + actor/critic MLP heads).

Strategy (per spec sharding hint): data-parallel over the env dim B across
8 NeuronCores (64 envs/core), weights replicated.  Inside each core the
recurrence runs in a transposed layout [channels, envs] so the per-step
hidden matmuls feed the tensor engine directly; the input projection (gi),
the actor/critic MLP and all transposes/DMA are overlapped with the
sequential GRU chain.

Self-contained: hardcodes all shapes; only depends on the platform's
concourse (Bass) library.
"""

import os
import sys

import numpy as np

for _p in ("/opt/trn_rl_repo", os.path.expanduser("~/.axon_site/_ro/trn_rl_repo")):
    if os.path.isdir(_p) and _p not in sys.path:
        sys.path.insert(0, _p)
        break

import concourse.bass as bass
import concourse.mybir as mybir
import concourse.tile as tile
from concourse import bacc
from concourse.masks import make_identity

T, B, OBS, H, A, L = 512, 512, 64, 64, 6, 64
N_CORES = 8
BL = B // N_CORES          # 64 envs per core
GS = 8                     # timesteps per group
COLS = GS * BL             # 512 columns (free dim) per group
H3 = 3 * H

F32 = mybir.dt.float32
AF = mybir.ActivationFunctionType
ALU = mybir.AluOpType

WEIGHT_KEYS = [
    "w_ih", "w_hh", "b_ih", "b_hh",
    "aw1", "ab1", "aw2", "ab2", "aw3", "ab3",
    "cw1", "cb1", "cw2", "cb2", "cw3", "cb3",
]


def build(nc, t_loc=T):
    """Emit the full per-core kernel into `nc` (a Bacc instance)."""
    from contextlib import ExitStack

    assert t_loc % GS == 0
    ng = t_loc // GS

    x_d = nc.dram_tensor("x", [t_loc, BL, OBS], F32, kind="ExternalInput")
    done_d = nc.dram_tensor("done", [t_loc, BL], F32, kind="ExternalInput")
    h0_d = nc.dram_tensor("h0", [BL, H], F32, kind="ExternalInput")
    wih_d = nc.dram_tensor("w_ih", [H3, OBS], F32, kind="ExternalInput")
    whh_d = nc.dram_tensor("w_hh", [H3, H], F32, kind="ExternalInput")
    bih_d = nc.dram_tensor("b_ih", [H3], F32, kind="ExternalInput")
    bhh_d = nc.dram_tensor("b_hh", [H3], F32, kind="ExternalInput")
    aw1_d = nc.dram_tensor("aw1", [L, H + OBS], F32, kind="ExternalInput")
    ab1_d = nc.dram_tensor("ab1", [L], F32, kind="ExternalInput")
    aw2_d = nc.dram_tensor("aw2", [L, L], F32, kind="ExternalInput")
    ab2_d = nc.dram_tensor("ab2", [L], F32, kind="ExternalInput")
    aw3_d = nc.dram_tensor("aw3", [A, L], F32, kind="ExternalInput")
    ab3_d = nc.dram_tensor("ab3", [A], F32, kind="ExternalInput")
    cw1_d = nc.dram_tensor("cw1", [L, H + OBS], F32, kind="ExternalInput")
    cb1_d = nc.dram_tensor("cb1", [L], F32, kind="ExternalInput")
    cw2_d = nc.dram_tensor("cw2", [L, L], F32, kind="ExternalInput")
    cb2_d = nc.dram_tensor("cb2", [L], F32, kind="ExternalInput")
    cw3_d = nc.dram_tensor("cw3", [1, L], F32, kind="ExternalInput")
    cb3_d = nc.dram_tensor("cb3", [1], F32, kind="ExternalInput")
    out_d = nc.dram_tensor("out", [t_loc, BL, A + 1], F32, kind="ExternalOutput")

    with tile.TileContext(nc) as tc, ExitStack() as ctx:
        wp = ctx.enter_context(tc.tile_pool(name="wp", bufs=1))
        ldp = ctx.enter_context(tc.tile_pool(name="ldp", bufs=2))
        catp = ctx.enter_context(tc.tile_pool(name="catp", bufs=3))
        xnp = ctx.enter_context(tc.tile_pool(name="xnp", bufs=2))
        drp = ctx.enter_context(tc.tile_pool(name="drp", bufs=2))
        mbp = ctx.enter_context(tc.tile_pool(name="mbp", bufs=3))
        small = ctx.enter_context(tc.tile_pool(name="small", bufs=3))
        tmlp = ctx.enter_context(tc.tile_pool(name="tmlp", bufs=2))
        onp = ctx.enter_context(tc.tile_pool(name="onp", bufs=2))

        przp = ctx.enter_context(tc.tile_pool(name="przp", bufs=2, space="PSUM"))
        pginp = ctx.enter_context(tc.tile_pool(name="pginp", bufs=2, space="PSUM"))
        pghnp = ctx.enter_context(tc.tile_pool(name="pghnp", bufs=2, space="PSUM"))
        pmisc = ctx.enter_context(tc.tile_pool(name="pmisc", bufs=2, space="PSUM"))

        ident = wp.tile([128, 128], F32, tag="ident")
        make_identity(nc, ident[:])

        def load_transposed(dram_ap, rows, cols, tag):
            """dram [rows, cols] -> sbuf tile [cols, rows] (features on partitions)."""
            dst = wp.tile([cols, rows], F32, tag=tag)
            r0 = 0
            while r0 < rows:
                rr = min(128, rows - r0)
                tmp = ldp.tile([128, 128], F32, tag="wtmp")
                nc.sync.dma_start(tmp[:rr, :cols], dram_ap[r0:r0 + rr, :])
                pt = pmisc.tile([128, COLS], F32, tag="pm")
                nc.tensor.transpose(pt[:cols, :rr], tmp[:rr, :cols], ident[:rr, :rr])
                nc.scalar.copy(dst[:, r0:r0 + rr], pt[:cols, :rr])
                r0 += rr
            return dst

        def load_col(dram_1d, n, tag, off=0, dst=None, dst_off=0):
            if dst is None:
                dst = wp.tile([max(n + dst_off, 1), 1], F32, tag=tag)
            nc.sync.dma_start(
                dst[dst_off:dst_off + n, :],
                dram_1d[off:off + n].rearrange("p -> p ()"),
            )
            return dst

        # --- weights / constants preprocessing (runs once, overlapped) ---
        w_ihT = load_transposed(wih_d[:], H3, OBS, "wihT")    # [64, 192]
        w_hhT = load_transposed(whh_d[:], H3, H, "whhT")      # [64, 192]
        h0T = load_transposed(h0_d[:], BL, H, "h0T")          # [64, 64] (h x b)

        lhsT1h = wp.tile([64, 128], F32, tag="lhsT1h")
        lhsT1x = wp.tile([64, 128], F32, tag="lhsT1x")
        for src, c0 in ((aw1_d, 0), (cw1_d, 64)):
            tmp = ldp.tile([128, 128], F32, tag="wtmp")
            nc.sync.dma_start(tmp[:L, :H + OBS], src[:, :])
            pt = pmisc.tile([128, COLS], F32, tag="pm")
            nc.tensor.transpose(pt[:H, :L], tmp[:L, 0:H], ident[:L, :L])
            nc.tensor.transpose(pt[:OBS, 128:128 + L], tmp[:L, H:H + OBS], ident[:L, :L])
            nc.scalar.copy(lhsT1h[:, c0:c0 + L], pt[:H, :L])
            nc.scalar.copy(lhsT1x[:, c0:c0 + L], pt[:OBS, 128:128 + L])

        lhsT2 = wp.tile([128, 128], F32, tag="lhsT2")
        nc.vector.memset(lhsT2[:], 0.0)
        for src, o in ((aw2_d, 0), (cw2_d, 64)):
            tmp = ldp.tile([128, 128], F32, tag="wtmp")
            nc.sync.dma_start(tmp[:L, :L], src[:, :])
            pt = pmisc.tile([128, COLS], F32, tag="pm")
            nc.tensor.transpose(pt[:L, :L], tmp[:L, :L], ident[:L, :L])
            nc.scalar.copy(lhsT2[o:o + L, o:o + L], pt[:L, :L])

        lhsT3 = wp.tile([128, A + 1], F32, tag="lhsT3")
        nc.vector.memset(lhsT3[:], 0.0)
        tmp = ldp.tile([128, 128], F32, tag="wtmp")
        nc.sync.dma_start(tmp[:A, :L], aw3_d[:, :])
        pt = pmisc.tile([128, COLS], F32, tag="pm")
        nc.tensor.transpose(pt[:L, :A], tmp[:A, :L], ident[:A, :A])
        nc.scalar.copy(lhsT3[:L, :A], pt[:L, :A])
        tmp = ldp.tile([128, 128], F32, tag="wtmp")
        nc.sync.dma_start(tmp[:1, :L], cw3_d[:, :])
        pt = pmisc.tile([128, COLS], F32, tag="pm")
        nc.tensor.transpose(pt[:L, :1], tmp[:1, :L], ident[:1, :1])
        nc.scalar.copy(lhsT3[64:64 + L, A:A + 1], pt[:L, :1])

        # biases
        bihc = load_col(bih_d, 128, "bihc")                   # b_ih[0:128]
        bhhc = load_col(bhh_d, 128, "bhhc")
        bias_r = wp.tile([64, 1], F32, tag="bias_r")
        nc.vector.tensor_add(bias_r[:], bihc[0:64, :], bhhc[0:64, :])
        bias_z = wp.tile([64, 1], F32, tag="bias_z")
        bihz = load_col(bih_d, 64, "bihz", off=64)
        bhhz = load_col(bhh_d, 64, "bhhz", off=64)
        nc.vector.tensor_add(bias_z[:], bihz[:], bhhz[:])
        negbz = wp.tile([64, 1], F32, tag="negbz")
        nc.vector.tensor_scalar_mul(negbz[:], bias_z[:], -1.0)
        b_ihn = load_col(bih_d, H, "b_ihn", off=128)          # [64,1]
        b_hhn = load_col(bhh_d, H, "b_hhn", off=128)          # [64,1]

        bias1 = wp.tile([128, 1], F32, tag="bias1")
        load_col(ab1_d, L, "bias1", dst=bias1, dst_off=0)
        load_col(cb1_d, L, "bias1", dst=bias1, dst_off=64)
        bias2 = wp.tile([128, 1], F32, tag="bias2")
        load_col(ab2_d, L, "bias2", dst=bias2, dst_off=0)
        load_col(cb2_d, L, "bias2", dst=bias2, dst_off=64)
        bias3 = wp.tile([A + 1, 1], F32, tag="bias3")
        load_col(ab3_d, A, "bias3", dst=bias3, dst_off=0)
        load_col(cb3_d, 1, "bias3", dst=bias3, dst_off=A)

        ones_row = wp.tile([1, BL], F32, tag="ones_row")
        nc.vector.memset(ones_row[:], 1.0)

        # --- steady-state group bodies ---
        def bulk(g):
            """x load + transpose, done -> reset-mask, gi preloads for group g."""
            hs = catp.tile([64, COLS], F32, tag="hs")
            xT = catp.tile([64, COLS], F32, tag="xT")
            xn = xnp.tile([128, GS // 2, OBS], F32, tag="xn")
            nc.sync.dma_start(
                xn[:],
                x_d[g * GS:(g + 1) * GS].rearrange("(k ph) b f -> (ph b) k f", ph=2),
            )
            ptx = pmisc.tile([128, COLS], F32, tag="pm")
            for k in range(GS // 2):
                nc.tensor.transpose(
                    ptx[:OBS, k * 128:(k + 1) * 128], xn[:, k, :], ident[:, :]
                )
            nc.scalar.copy(xT[:], ptx[:OBS, :])

            dr = drp.tile([1, COLS], F32, tag="dr")
            nc.sync.dma_start(
                dr[:], done_d[g * GS:(g + 1) * GS].rearrange("t b -> () (t b)")
            )
            pmb = pmisc.tile([128, COLS], F32, tag="pm")
            nc.tensor.matmul(pmb[:BL, :], ones_row[:], dr[:], start=True, stop=True)
            mb = mbp.tile([BL, COLS], F32, tag="mb")
            nc.scalar.activation(mb[:], pmb[:BL, :], AF.Identity, scale=-1.0, bias=1.0)

            prz = przp.tile([128, COLS], F32, tag="prz")
            nc.tensor.matmul(
                prz[:], w_ihT[:, 0:128], xT[:],
                start=True, stop=False, skip_group_check=True,
            )
            pgin = pginp.tile([BL, COLS], F32, tag="pgin")
            nc.tensor.matmul(
                pgin[:], w_ihT[:, 128:H3], xT[:], start=True, stop=True
            )
            return dict(hs=hs, xT=xT, mb=mb, prz=prz, pgin=pgin)

        state = {}

        def chain(g, refs, refs_next):
            prz, pgin, mb, hs = refs["prz"], refs["pgin"], refs["mb"], refs["hs"]
            for s in range(GS):
                t = g * GS + s
                cs = bass.ts(s, BL)
                mh = state["mh"]
                pghn = pghnp.tile([BL, BL], F32, tag="pghn")
                nc.tensor.matmul(
                    pghn[:], w_hhT[:, 128:H3], mh[:], start=True, stop=True
                )
                nc.tensor.matmul(
                    prz[:, cs], w_hhT[:, 0:128], mh[:],
                    start=False, stop=(s == GS - 1), skip_group_check=True,
                )
                r_t = small.tile([BL, BL], F32, tag="r_t")
                nc.scalar.activation(r_t[:], prz[0:64, cs], AF.Sigmoid, bias=bias_r[:])
                z_t = small.tile([BL, BL], F32, tag="z_t")
                nc.scalar.activation(z_t[:], prz[64:128, cs], AF.Sigmoid, bias=bias_z[:])
                u = small.tile([BL, BL], F32, tag="u")
                nc.scalar.activation(
                    u[:], prz[64:128, cs], AF.Sigmoid, scale=-1.0, bias=negbz[:]
                )
                zm = small.tile([BL, BL], F32, tag="zm")
                nc.gpsimd.tensor_mul(zm[:], z_t[:], mh[:])
                p = small.tile([BL, BL], F32, tag="p")
                nc.vector.scalar_tensor_tensor(
                    p[:], pghn[:], b_hhn[:], r_t[:], ALU.add, ALU.mult
                )
                q = small.tile([BL, BL], F32, tag="q")
                nc.vector.tensor_add(q[:], p[:], pgin[:, cs])
                n = small.tile([BL, BL], F32, tag="n")
                nc.scalar.activation(n[:], q[:], AF.Tanh, bias=b_ihn[:])
                v = small.tile([BL, BL], F32, tag="v")
                nc.vector.tensor_mul(v[:], n[:], u[:])
                nc.vector.tensor_add(hs[:, cs], v[:], zm[:])
                if t < t_loc - 1:
                    mh2 = small.tile([BL, BL], F32, tag="mh")
                    if s == GS - 1:
                        mbn = refs_next["mb"][:, 0:BL]
                    else:
                        mbn = mb[:, bass.ts(s + 1, BL)]
                    nc.vector.tensor_mul(mh2[:], hs[:, cs], mbn)
                    state["mh"] = mh2

        def head(g, refs):
            hs, xT = refs["hs"], refs["xT"]
            p1 = pmisc.tile([128, COLS], F32, tag="pm")
            nc.tensor.matmul(p1[:], lhsT1h[:], hs[:], start=True, stop=False,
                             skip_group_check=True)
            nc.tensor.matmul(p1[:], lhsT1x[:], xT[:], start=False, stop=True,
                             skip_group_check=True)
            t1 = tmlp.tile([128, COLS], F32, tag="t1")
            nc.scalar.activation(t1[:], p1[:], AF.Tanh, bias=bias1[:])
            p2 = pmisc.tile([128, COLS], F32, tag="pm")
            nc.tensor.matmul(p2[:], lhsT2[:], t1[:], start=True, stop=True)
            t2 = tmlp.tile([128, COLS], F32, tag="t2")
            nc.scalar.activation(t2[:], p2[:], AF.Tanh, bias=bias2[:])
            p3 = pmisc.tile([128, COLS], F32, tag="pm")
            nc.tensor.matmul(p3[:A + 1, :], lhsT3[:], t2[:], start=True, stop=True)
            o7 = tmlp.tile([A + 1, COLS], F32, tag="o7")
            nc.scalar.activation(o7[:], p3[:A + 1, :], AF.Identity, bias=bias3[:])

            po = pmisc.tile([128, GS // 2, A + 1], F32, tag="pm")
            for k in range(GS // 2):
                nc.tensor.transpose(
                    po[:, k, :], o7[:, k * 128:(k + 1) * 128], ident[:A + 1, :A + 1]
                )
            on = onp.tile([128, GS // 2, A + 1], F32, tag="on")
            nc.vector.tensor_copy(on[:], po[:])
            nc.sync.dma_start(
                out_d[g * GS:(g + 1) * GS].rearrange("(k ph) b j -> (ph b) k j", ph=2),
                on[:],
            )

        refs = bulk(0)
        mh0 = small.tile([BL, BL], F32, tag="mh")
        nc.vector.tensor_mul(mh0[:], h0T[:], refs["mb"][:, 0:BL])
        state["mh"] = mh0
        for g in range(1, ng):
            refs_next = bulk(g)
            chain(g - 1, refs, refs_next)
            head(g - 1, refs)
            refs = refs_next
        chain(ng - 1, refs, None)
        head(ng - 1, refs)

    return nc


_BUILT = {}


def get_built(t_loc=T):
    if t_loc not in _BUILT:
        nc = bacc.Bacc(None, target_bir_lowering=False)
        build(nc, t_loc)
        nc.compile()
        _BUILT[t_loc] = nc
    return _BUILT[t_loc]


def shard_inputs(inputs, t_loc=T):
    """Full inputs dict -> list of 8 per-core input maps."""
    x = np.ascontiguousarray(np.asarray(inputs["x"], np.float32)).reshape(t_loc, B, OBS)
    done = np.ascontiguousarray(np.asarray(inputs["done"], np.float32)).reshape(t_loc, B)
    h0 = np.ascontiguousarray(np.asarray(inputs["gru_state"], np.float32)).reshape(B, H)
    common = {
        k: np.ascontiguousarray(np.asarray(inputs[k], np.float32))
        for k in WEIGHT_KEYS
    }
    in_maps = []
    for c in range(N_CORES):
        sl = slice(c * BL, (c + 1) * BL)
        m = dict(common)
        m["x"] = np.ascontiguousarray(x[:, sl, :])
        m["done"] = np.ascontiguousarray(done[:, sl])
        m["h0"] = np.ascontiguousarray(h0[sl, :])
        in_maps.append(m)
    return in_maps


def assemble_output(per_core_outs, t_loc=T):
    outs = [np.asarray(o, np.float32).reshape(t_loc, BL, A + 1) for o in per_core_outs]
    full = np.stack(outs, axis=1).reshape(t_loc, B, A + 1)
    return np.ascontiguousarray(full.reshape(t_loc * B, A + 1))


def run_on_hw(inputs, t_loc=T, trace=False, **kw):
    from concourse.bass_utils import run_bass_kernel_spmd

    nc = get_built(t_loc)
    in_maps = shard_inputs(inputs, t_loc)
    res = run_bass_kernel_spmd(
        nc, in_maps, core_ids=list(range(N_CORES)), trace=trace, **kw
    )
    out = assemble_output([r["out"] for r in res.results], t_loc)
    return out, res


def kernel(**inputs):
    out, _ = run_on_hw(inputs)
    return out



# revision 13
# speedup vs baseline: 3.1827x; 3.1827x over previous
"""Trainium2 Bass kernel for nn_GruAgent (GRU + actor/critic MLP heads).

Strategy: the `done` flags reset the GRU state, cutting every env's
512-step timeline into independent segments (mean length ~20, max ~184
for the reference inputs).  On the host we bin-pack all segments into
8*C lanes of length N (~195), which shortens the sequential recurrence
from 512 chain steps to ~195 while widening each step's tiles.  The
per-core kernel runs the recurrence in a transposed layout
[features, lanes] with bf16 matmuls/gates, the input projection and the
actor/critic heads overlapped with the sequential chain.  The host
permutes x/done in, un-permutes the outputs; this is exact (resets make
segments independent), not an approximation.

Self-contained: hardcodes all shapes; only depends on the platform's
concourse (Bass) library.
"""

import math
import os
import sys

import numpy as np

for _p in ("/opt/trn_rl_repo", os.path.expanduser("~/.axon_site/_ro/trn_rl_repo")):
    if os.path.isdir(_p) and _p not in sys.path:
        sys.path.insert(0, _p)
        break

import ml_dtypes

import concourse.bass as bass
import concourse.mybir as mybir
import concourse.tile as tile
from concourse import bacc

T, B, OBS, H, A, L = 512, 512, 64, 64, 6, 64
N_CORES = 8
C = 170                     # lanes (columns) per core
GS = 3                      # chain steps per group (GS*C <= 512 psum fp32)
COLS = GS * C               # 510
H3 = 3 * H
NLANES = N_CORES * C        # 1360
AO = A + 1                  # 7 outputs (6 logits + 1 value)

F32 = mybir.dt.float32
BF16 = mybir.dt.bfloat16
AF = mybir.ActivationFunctionType
ALU = mybir.AluOpType
BF = ml_dtypes.bfloat16

WEIGHT_KEYS = [
    "w_ih", "w_hh", "b_ih", "b_hh",
    "aw1", "ab1", "aw2", "ab2", "aw3", "ab3",
    "cw1", "cb1", "cw2", "cb2", "cw3", "cb3",
]


# --------------------------------------------------------------------------
# device kernel
# --------------------------------------------------------------------------

def build(nc, ng):
    """Emit the per-core kernel for ng groups (N = ng*GS chain steps)."""
    from contextlib import ExitStack

    n_steps = ng * GS

    xT_d = nc.dram_tensor("xT", [ng, OBS, COLS], BF16, kind="ExternalInput")
    mb_d = nc.dram_tensor("mb", [ng, H, COLS], BF16, kind="ExternalInput")
    h0T_d = nc.dram_tensor("h0T", [H, C], BF16, kind="ExternalInput")
    wih_d = nc.dram_tensor("w_ihT", [OBS, H3], BF16, kind="ExternalInput")
    whh_d = nc.dram_tensor("w_hhT", [H, H3], BF16, kind="ExternalInput")
    l1h_d = nc.dram_tensor("l1h", [64, 128], BF16, kind="ExternalInput")
    l1x_d = nc.dram_tensor("l1x", [64, 128], BF16, kind="ExternalInput")
    l2_d = nc.dram_tensor("l2", [128, 128], BF16, kind="ExternalInput")
    l3_d = nc.dram_tensor("l3", [128, AO], BF16, kind="ExternalInput")
    brz_d = nc.dram_tensor("b_rz", [128, 1], F32, kind="ExternalInput")
    bhhn_d = nc.dram_tensor("b_hhn", [H, 1], F32, kind="ExternalInput")
    bihn_d = nc.dram_tensor("b_ihn", [H, 1], F32, kind="ExternalInput")
    b1_d = nc.dram_tensor("b1", [128, 1], F32, kind="ExternalInput")
    b2_d = nc.dram_tensor("b2", [128, 1], F32, kind="ExternalInput")
    b3_d = nc.dram_tensor("b3", [AO, 1], F32, kind="ExternalInput")
    out_d = nc.dram_tensor("out", [ng, AO, COLS], F32, kind="ExternalOutput")

    with tile.TileContext(nc) as tc, ExitStack() as ctx:
        wp = ctx.enter_context(tc.tile_pool(name="wp", bufs=1))
        catp = ctx.enter_context(tc.tile_pool(name="catp", bufs=3))
        mbp = ctx.enter_context(tc.tile_pool(name="mbp", bufs=3))
        gip = ctx.enter_context(tc.tile_pool(name="gip", bufs=2))
        gatep = ctx.enter_context(tc.tile_pool(name="gatep", bufs=2))
        smallp = ctx.enter_context(tc.tile_pool(name="smallp", bufs=2))
        tmlp = ctx.enter_context(tc.tile_pool(name="tmlp", bufs=2))
        o7p = ctx.enter_context(tc.tile_pool(name="o7p", bufs=2))

        przp = ctx.enter_context(tc.tile_pool(name="przp", bufs=2, space="PSUM"))
        pginp = ctx.enter_context(tc.tile_pool(name="pginp", bufs=2, space="PSUM"))
        pghnp = ctx.enter_context(tc.tile_pool(name="pghnp", bufs=2, space="PSUM"))
        phd = ctx.enter_context(tc.tile_pool(name="phd", bufs=2, space="PSUM"))

        # ---- weights / biases (host-prepared, straight DMA loads) ----
        def load(dram, shape, dt, tag):
            t = wp.tile(shape, dt, tag=tag)
            nc.sync.dma_start(t[:], dram[:])
            return t

        wihT = load(wih_d, [OBS, H3], BF16, "wihT")
        whhT = load(whh_d, [H, H3], BF16, "whhT")
        h0T = load(h0T_d, [H, C], BF16, "h0T")
        l1h = load(l1h_d, [64, 128], BF16, "l1h")
        l1x = load(l1x_d, [64, 128], BF16, "l1x")
        l2 = load(l2_d, [128, 128], BF16, "l2")
        l3 = load(l3_d, [128, AO], BF16, "l3")
        b_r = wp.tile([H, 1], F32, tag="b_r")
        nc.sync.dma_start(b_r[:], brz_d[0:64])
        b_z = wp.tile([H, 1], F32, tag="b_z")
        nc.sync.dma_start(b_z[:], brz_d[64:128])
        b_hhn = load(bhhn_d, [H, 1], F32, "b_hhn")
        b_ihn = load(bihn_d, [H, 1], F32, "b_ihn")
        b1 = load(b1_d, [128, 1], F32, "b1")
        b2 = load(b2_d, [128, 1], F32, "b2")
        b3 = load(b3_d, [AO, 1], F32, "b3")

        def bulk(g):
            """x + mask loads, input-projection preloads for group g."""
            xT = catp.tile([OBS, COLS], BF16, tag="xT")
            nc.sync.dma_start(xT[:], xT_d[g])
            hsg = catp.tile([H, COLS], BF16, tag="hsg")
            mbt = mbp.tile([H, COLS], BF16, tag="mb")
            nc.sync.dma_start(mbt[:], mb_d[g])
            prz = przp.tile([128, COLS], F32, tag="prz")
            nc.tensor.matmul(
                prz[:], wihT[:, 0:128], xT[:],
                start=True, stop=False, skip_group_check=True,
            )
            pgin = pginp.tile([H, COLS], F32, tag="pgin")
            nc.tensor.matmul(
                pgin[:], wihT[:, 128:H3], xT[:], start=True, stop=True
            )
            ginS = gip.tile([H, COLS], BF16, tag="ginS")
            nc.vector.tensor_copy(ginS[:], pgin[:])
            return dict(xT=xT, hsg=hsg, mb=mbt, prz=prz, ginS=ginS)

        state = {}

        def chain(g, refs, refs_next):
            hsg, mbt, prz, ginS = refs["hsg"], refs["mb"], refs["prz"], refs["ginS"]
            for s in range(GS):
                t = g * GS + s
                cs = bass.ts(s, C)
                mh = state["mh"]
                nc.tensor.matmul(
                    prz[:, cs], whhT[:, 0:128], mh[:],
                    start=False, stop=(s == GS - 1), skip_group_check=True,
                )
                pghn = pghnp.tile([H, C], F32, tag="pghn")
                nc.tensor.matmul(
                    pghn[:], whhT[:, 128:H3], mh[:], start=True, stop=True
                )
                r = gatep.tile([H, C], BF16, tag="r")
                nc.scalar.activation(r[:], prz[0:64, cs], AF.Sigmoid, bias=b_r[:])
                z = gatep.tile([H, C], BF16, tag="z")
                nc.scalar.activation(z[:], prz[64:128, cs], AF.Sigmoid, bias=b_z[:])
                zm1 = smallp.tile([H, C], BF16, tag="zm1")
                nc.gpsimd.tensor_mul(zm1[:], z[:], mh[:])
                p = smallp.tile([H, C], BF16, tag="p")
                nc.vector.scalar_tensor_tensor(
                    p[:], pghn[:], b_hhn[:], r[:], ALU.add, ALU.mult
                )
                q = smallp.tile([H, C], BF16, tag="q")
                nc.vector.tensor_add(q[:], p[:], ginS[:, cs])
                n = smallp.tile([H, C], BF16, tag="n")
                nc.scalar.activation(n[:], q[:], AF.Tanh, bias=b_ihn[:])
                vb = smallp.tile([H, C], BF16, tag="vb")
                nc.vector.scalar_tensor_tensor(
                    vb[:], z[:], 1.0, n[:], ALU.subtract, ALU.mult
                )
                nc.vector.tensor_sub(hsg[:, cs], zm1[:], vb[:])
                if t < n_steps - 1:
                    mh2 = smallp.tile([H, C], BF16, tag="mh")
                    if s == GS - 1:
                        mbn = refs_next["mb"][:, 0:C]
                    else:
                        mbn = mbt[:, bass.ts(s + 1, C)]
                    nc.vector.tensor_mul(mh2[:], hsg[:, cs], mbn)
                    state["mh"] = mh2

        def head(g, refs):
            hsg, xT = refs["hsg"], refs["xT"]
            p1 = phd.tile([128, COLS], F32, tag="ph")
            nc.tensor.matmul(p1[:], l1h[:], hsg[:], start=True, stop=False,
                             skip_group_check=True)
            nc.tensor.matmul(p1[:], l1x[:], xT[:], start=False, stop=True,
                             skip_group_check=True)
            t1 = tmlp.tile([128, COLS], BF16, tag="t1")
            nc.scalar.activation(t1[:], p1[:], AF.Tanh, bias=b1[:])
            p2 = phd.tile([128, COLS], F32, tag="ph")
            nc.tensor.matmul(p2[:], l2[:], t1[:], start=True, stop=True)
            t2 = tmlp.tile([128, COLS], BF16, tag="t2")
            nc.scalar.activation(t2[:], p2[:], AF.Tanh, bias=b2[:])
            p3 = phd.tile([128, COLS], F32, tag="ph")
            nc.tensor.matmul(p3[:AO, :], l3[:], t2[:], start=True, stop=True)
            o7 = o7p.tile([AO, COLS], F32, tag="o7")
            nc.vector.tensor_scalar_add(o7[:], p3[:AO, :], b3[:])
            nc.sync.dma_start(out_d[g], o7[:])

        refs = bulk(0)
        mh0 = smallp.tile([H, C], BF16, tag="mh")
        nc.vector.tensor_mul(mh0[:], h0T[:], refs["mb"][:, 0:C])
        state["mh"] = mh0
        for g in range(1, ng):
            refs_next = bulk(g)
            chain(g - 1, refs, refs_next)
            head(g - 1, refs)
            refs = refs_next
        chain(ng - 1, refs, None)
        head(ng - 1, refs)

    return nc


_BUILT = {}


def get_built(ng):
    if ng not in _BUILT:
        nc = bacc.Bacc(None, target_bir_lowering=False)
        build(nc, ng)
        nc.compile()
        _BUILT[ng] = nc
    return _BUILT[ng]


# --------------------------------------------------------------------------
# host-side packing
# --------------------------------------------------------------------------

def _enumerate_segments(done2):
    """done2 [T,B] -> (seg_env, seg_t0, seg_len) with cuts at done==1.0."""
    starts = done2 == 1.0
    starts[0, :] = True
    nseg_per_env = starts.sum(axis=0)
    seg_env = np.repeat(np.arange(done2.shape[1]), nseg_per_env)
    env_idx, t_idx = np.nonzero(starts.T)
    seg_t0 = t_idx
    # length = next start - this start (within env)
    seg_len = np.empty(len(seg_t0), np.int64)
    pos = 0
    Tn = done2.shape[0]
    for b, k in enumerate(nseg_per_env):
        ts = seg_t0[pos:pos + k]
        seg_len[pos:pos + k - 1] = np.diff(ts)
        seg_len[pos + k - 1] = Tn - ts[-1]
        pos += k
    return seg_env, seg_t0, seg_len


def _pack(done2, h0_nonzero):
    """Bin-pack segments into NLANES lanes.  Returns (N, lane, off) per seg."""
    seg_env, seg_t0, seg_len = _enumerate_segments(done2)
    nseg = len(seg_len)
    total = int(seg_len.sum())
    cap = max(int(seg_len.max()), math.ceil(total / NLANES))
    cap = ((cap + GS - 1) // GS) * GS

    while True:
        rem = np.full(NLANES, cap, np.int64)
        lane = np.full(nseg, -1, np.int64)
        off = np.zeros(nseg, np.int64)
        ok = True
        if h0_nonzero:
            # t=0 segments that continue from h0 must sit at a lane start
            pin = np.nonzero((seg_t0 == 0) & (done2[0, seg_env] != 1.0))[0]
            if len(pin) > NLANES:
                raise RuntimeError("too many h0 segments")
            for j, i in enumerate(pin):
                if seg_len[i] > cap:
                    ok = False
                    break
                lane[i] = j
                off[i] = 0
                rem[j] = cap - seg_len[i]
        if ok:
            order = np.argsort(-seg_len, kind="stable")
            for i in order:
                if lane[i] >= 0:
                    continue
                Lg = seg_len[i]
                j = int(np.argmax(rem >= Lg))
                if rem[j] < Lg:
                    ok = False
                    break
                lane[i] = j
                off[i] = cap - rem[j]
                rem[j] -= Lg
        if ok:
            return cap, seg_env, seg_t0, seg_len, lane, off
        cap += GS


def _prepare(inputs):
    """Host-side pack + permute.  Returns (ng, in_maps, gather_idx)."""
    x = np.ascontiguousarray(np.asarray(inputs["x"], np.float32)).reshape(T, B, OBS)
    done2 = np.ascontiguousarray(
        np.asarray(inputs["done"], np.float32)
    ).reshape(T, B)
    h0 = np.ascontiguousarray(
        np.asarray(inputs["gru_state"], np.float32)
    ).reshape(B, H)
    h0_nonzero = bool(np.any(h0))

    N, seg_env, seg_t0, seg_len, seg_lane, seg_off = _pack(done2, h0_nonzero)
    ng = N // GS

    # flat (src slot) -> (dst slot) index arrays
    reps = seg_len
    src_env = np.repeat(seg_env, reps)
    within = np.concatenate([np.arange(l) for l in seg_len])
    src_t = np.repeat(seg_t0, reps) + within
    dst_lane = np.repeat(seg_lane, reps)
    dst_n = np.repeat(seg_off, reps) + within

    # packed done: copy source done values; padding slots = 1 (reset)
    donep = np.ones((N, NLANES), np.float32)
    donep[dst_n, dst_lane] = done2[src_t, src_env]
    if not h0_nonzero:
        # zero h0 == reset; break any dependence on prior lane garbage
        startmask = within == 0
        donep[dst_n[startmask], dst_lane[startmask]] = 1.0
    mbp_ = (1.0 - donep).astype(BF)                       # [N, NLANES]

    xp = np.zeros((N, NLANES, OBS), BF)
    xp[dst_n, dst_lane] = x[src_t, src_env].astype(BF)

    h0T_all = np.zeros((H, NLANES), BF)
    if h0_nonzero:
        first = within == 0
        fl = dst_lane[first]
        fe = src_env[first]
        fn = dst_n[first]
        sel = fn == 0
        h0T_all[:, fl[sel]] = h0[fe[sel]].T.astype(BF)

    # weights, host-transformed
    w_ih = np.asarray(inputs["w_ih"], np.float32)
    w_hh = np.asarray(inputs["w_hh"], np.float32)
    b_ih = np.asarray(inputs["b_ih"], np.float32)
    b_hh = np.asarray(inputs["b_hh"], np.float32)
    l1 = np.concatenate(
        [np.asarray(inputs["aw1"], np.float32).T,
         np.asarray(inputs["cw1"], np.float32).T], axis=1)       # [128,128]
    l2 = np.zeros((128, 128), np.float32)
    l2[0:64, 0:64] = np.asarray(inputs["aw2"], np.float32).T
    l2[64:128, 64:128] = np.asarray(inputs["cw2"], np.float32).T
    l3 = np.zeros((128, AO), np.float32)
    l3[0:64, 0:A] = np.asarray(inputs["aw3"], np.float32).T
    l3[64:128, A:AO] = np.asarray(inputs["cw3"], np.float32).T
    brz = (b_ih[0:128] + b_hh[0:128]).reshape(128, 1)
    bhhn = b_hh[128:H3].reshape(H, 1)
    bihn = b_ih[128:H3].reshape(H, 1)
    b1 = np.concatenate(
        [np.asarray(inputs["ab1"], np.float32),
         np.asarray(inputs["cb1"], np.float32)]).reshape(128, 1)
    b2 = np.concatenate(
        [np.asarray(inputs["ab2"], np.float32),
         np.asarray(inputs["cb2"], np.float32)]).reshape(128, 1)
    b3 = np.concatenate(
        [np.asarray(inputs["ab3"], np.float32),
         np.asarray(inputs["cb3"], np.float32)]).reshape(AO, 1)

    common = {
        "w_ihT": np.ascontiguousarray(w_ih.T.astype(BF)),
        "w_hhT": np.ascontiguousarray(w_hh.T.astype(BF)),
        "l1h": np.ascontiguousarray(l1[0:64].astype(BF)),
        "l1x": np.ascontiguousarray(l1[64:128].astype(BF)),
        "l2": l2.astype(BF), "l3": l3.astype(BF),
        "b_rz": brz, "b_hhn": bhhn, "b_ihn": bihn,
        "b1": b1, "b2": b2, "b3": b3,
    }

    in_maps = []
    for c in range(N_CORES):
        sl = slice(c * C, (c + 1) * C)
        # [N, C, OBS] -> [ng, OBS, GS*C] with column order (s, lane)
        xc = xp[:, sl, :].reshape(ng, GS, C, OBS).transpose(0, 3, 1, 2)
        mc = mbp_[:, sl].reshape(ng, GS, C)
        mcb = np.broadcast_to(mc[:, None, :, :], (ng, H, GS, C))
        m = dict(common)
        m["xT"] = np.ascontiguousarray(xc.reshape(ng, OBS, COLS))
        m["mb"] = np.ascontiguousarray(mcb.reshape(ng, H, COLS))
        m["h0T"] = np.ascontiguousarray(h0T_all[:, sl])
        in_maps.append(m)

    gather = (src_t, src_env, dst_n, dst_lane, N, ng)
    return ng, in_maps, gather


def _assemble(per_core_outs, gather):
    src_t, src_env, dst_n, dst_lane, N, ng = gather
    # per-core out [ng, AO, COLS] -> [N, C, AO]
    packed = np.concatenate(
        [
            np.asarray(o, np.float32)
            .reshape(ng, AO, GS, C)
            .transpose(0, 2, 3, 1)
            .reshape(N, C, AO)
            for o in per_core_outs
        ],
        axis=1,
    )                                                     # [N, NLANES, AO]
    full = np.empty((T * B, AO), np.float32)
    full[src_t * B + src_env] = packed[dst_n, dst_lane]
    return full


def run_on_hw(inputs, trace=False, **kw):
    from concourse.bass_utils import run_bass_kernel_spmd

    ng, in_maps, gather = _prepare(inputs)
    nc = get_built(ng)
    res = run_bass_kernel_spmd(
        nc, in_maps, core_ids=list(range(N_CORES)), trace=trace, **kw
    )
    out = _assemble([r["out"] for r in res.results], gather)
    return out, res


def kernel(**inputs):
    out, _ = run_on_hw(inputs)
    return out


# revision 22
# speedup vs baseline: 3.2277x; 1.0142x over previous
"""Trainium2 Bass kernel for nn_GruAgent (GRU + actor/critic MLP heads).

Strategy: the `done` flags reset the GRU state, cutting every env's
512-step timeline into independent segments (mean length ~20, max ~184
for the reference inputs).  On the host we bin-pack all segments into
8*C lanes of length N (~195), which shortens the sequential recurrence
from 512 chain steps to ~195 while widening each step's tiles.  The
per-core kernel runs the recurrence in a transposed layout
[features, lanes] with bf16 matmuls/gates, the input projection and the
actor/critic heads overlapped with the sequential chain.  The host
permutes x/done in, un-permutes the outputs; this is exact (resets make
segments independent), not an approximation.

Self-contained: hardcodes all shapes; only depends on the platform's
concourse (Bass) library.
"""

import math
import os
import sys

import numpy as np

for _p in ("/opt/trn_rl_repo", os.path.expanduser("~/.axon_site/_ro/trn_rl_repo")):
    if os.path.isdir(_p) and _p not in sys.path:
        sys.path.insert(0, _p)
        break

import ml_dtypes

import concourse.bass as bass
import concourse.mybir as mybir
import concourse.tile as tile
from concourse import bacc

T, B, OBS, H, A, L = 512, 512, 64, 64, 6, 64
N_CORES = 8
C = 170                     # lanes (columns) per core
GS = 3                      # chain steps per group (GS*C <= 512 psum fp32)
COLS = GS * C               # 510
H3 = 3 * H
NLANES = N_CORES * C        # 1360
AO = A + 1                  # 7 outputs (6 logits + 1 value)

F32 = mybir.dt.float32
BF16 = mybir.dt.bfloat16
AF = mybir.ActivationFunctionType
ALU = mybir.AluOpType
BF = ml_dtypes.bfloat16

WEIGHT_KEYS = [
    "w_ih", "w_hh", "b_ih", "b_hh",
    "aw1", "ab1", "aw2", "ab2", "aw3", "ab3",
    "cw1", "cb1", "cw2", "cb2", "cw3", "cb3",
]


# --------------------------------------------------------------------------
# device kernel
# --------------------------------------------------------------------------

def build(nc, ng):
    """Emit the per-core kernel for ng groups (N = ng*GS chain steps)."""
    from contextlib import ExitStack

    n_steps = ng * GS

    xT_d = nc.dram_tensor("xT", [ng, OBS, COLS], BF16, kind="ExternalInput")
    mb_d = nc.dram_tensor("mb", [ng, H, COLS], BF16, kind="ExternalInput")
    h0T_d = nc.dram_tensor("h0T", [H, C], BF16, kind="ExternalInput")
    wih_d = nc.dram_tensor("w_ihT", [OBS, H3], BF16, kind="ExternalInput")
    whh_d = nc.dram_tensor("w_hhT", [H, H3], BF16, kind="ExternalInput")
    l1h_d = nc.dram_tensor("l1h", [64, 128], BF16, kind="ExternalInput")
    l1x_d = nc.dram_tensor("l1x", [64, 128], BF16, kind="ExternalInput")
    l2_d = nc.dram_tensor("l2", [128, 128], BF16, kind="ExternalInput")
    l3_d = nc.dram_tensor("l3", [128, AO], BF16, kind="ExternalInput")
    brz_d = nc.dram_tensor("b_rz", [128, 1], F32, kind="ExternalInput")
    bhhn_d = nc.dram_tensor("b_hhn", [H, 1], F32, kind="ExternalInput")
    bihn_d = nc.dram_tensor("b_ihn", [H, 1], F32, kind="ExternalInput")
    b1_d = nc.dram_tensor("b1", [128, 1], F32, kind="ExternalInput")
    b2_d = nc.dram_tensor("b2", [128, 1], F32, kind="ExternalInput")
    b3_d = nc.dram_tensor("b3", [AO, 1], F32, kind="ExternalInput")
    b3r_d = nc.dram_tensor("b3r", [1, AO], BF16, kind="ExternalInput")
    out_d = nc.dram_tensor("out", [ng, AO, COLS], F32, kind="ExternalOutput")

    with tile.TileContext(nc) as tc, ExitStack() as ctx:
        wp = ctx.enter_context(tc.tile_pool(name="wp", bufs=1))
        catp = ctx.enter_context(tc.tile_pool(name="catp", bufs=3))
        mbp = ctx.enter_context(tc.tile_pool(name="mbp", bufs=3))

        gatep = ctx.enter_context(tc.tile_pool(name="gatep", bufs=2))
        smallp = ctx.enter_context(tc.tile_pool(name="smallp", bufs=2))
        tmlp = ctx.enter_context(tc.tile_pool(name="tmlp", bufs=2))
        o7p = ctx.enter_context(tc.tile_pool(name="o7p", bufs=2))

        przp = ctx.enter_context(tc.tile_pool(name="przp", bufs=2, space="PSUM"))
        pginp = ctx.enter_context(tc.tile_pool(name="pginp", bufs=2, space="PSUM"))
        pghnp = ctx.enter_context(tc.tile_pool(name="pghnp", bufs=1, space="PSUM"))
        phd = ctx.enter_context(tc.tile_pool(name="phd", bufs=2, space="PSUM"))

        # ---- weights / biases (host-prepared, straight DMA loads) ----
        def load(dram, shape, dt, tag):
            t = wp.tile(shape, dt, tag=tag)
            nc.sync.dma_start(t[:], dram[:])
            return t

        wihT = load(wih_d, [OBS, H3], BF16, "wihT")
        whhT = load(whh_d, [H, H3], BF16, "whhT")
        h0T = load(h0T_d, [H, C], BF16, "h0T")
        l1h = load(l1h_d, [64, 128], BF16, "l1h")
        l1x = load(l1x_d, [64, 128], BF16, "l1x")
        l2 = load(l2_d, [128, 128], BF16, "l2")
        l3 = load(l3_d, [128, AO], BF16, "l3")
        b_r = wp.tile([H, 1], F32, tag="b_r")
        nc.sync.dma_start(b_r[:], brz_d[0:64])
        b_z = wp.tile([H, 1], F32, tag="b_z")
        nc.sync.dma_start(b_z[:], brz_d[64:128])
        b_hhn = load(bhhn_d, [H, 1], F32, "b_hhn")
        b_ihn = load(bihn_d, [H, 1], F32, "b_ihn")
        b1 = load(b1_d, [128, 1], F32, "b1")
        b2 = load(b2_d, [128, 1], F32, "b2")
        b3r = load(b3r_d, [1, AO], BF16, "b3r")
        ones_row = wp.tile([1, COLS], BF16, tag="ones_row")
        nc.vector.memset(ones_row[:], 1.0)

        def bulk(g):
            """x + mask loads, input-projection preloads for group g."""
            xT = catp.tile([OBS, COLS], BF16, tag="xT")
            nc.sync.dma_start(xT[:], xT_d[g])
            hsg = catp.tile([H, COLS], BF16, tag="hsg")
            mbt = mbp.tile([H, COLS], BF16, tag="mb")
            nc.sync.dma_start(mbt[:], mb_d[g])
            prz = przp.tile([128, COLS], F32, tag="prz")
            nc.tensor.matmul(
                prz[:], wihT[:, 0:128], xT[:],
                start=True, stop=False, skip_group_check=True,
            )
            pgin = pginp.tile([H, COLS], F32, tag="pgin")
            nc.tensor.matmul(
                pgin[:], wihT[:, 128:H3], xT[:], start=True, stop=True
            )
            return dict(xT=xT, hsg=hsg, mb=mbt, prz=prz, pgin=pgin)

        state = {}

        def chain(g, refs, refs_next, pieces=None):
            hsg, mbt, prz, pgin = refs["hsg"], refs["mb"], refs["prz"], refs["pgin"]
            for s in range(GS):
                t = g * GS + s
                cs = bass.ts(s, C)
                mh = state["mh"]
                nc.tensor.matmul(
                    prz[:, cs], whhT[:, 0:128], mh[:],
                    start=False, stop=(s == GS - 1), skip_group_check=True,
                )
                pghn = pghnp.tile([H, C], F32, tag="pghn")
                nc.tensor.matmul(
                    pghn[:], whhT[:, 128:H3], mh[:], start=True, stop=True
                )
                r = gatep.tile([H, C], BF16, tag="r")
                nc.scalar.activation(r[:], prz[0:64, cs], AF.Sigmoid, bias=b_r[:])
                z = gatep.tile([H, C], BF16, tag="z")
                nc.scalar.activation(z[:], prz[64:128, cs], AF.Sigmoid, bias=b_z[:])
                zm1 = smallp.tile([H, C], BF16, tag="zm1")
                nc.gpsimd.tensor_mul(zm1[:], z[:], mh[:])
                p = smallp.tile([H, C], BF16, tag="p")
                nc.vector.scalar_tensor_tensor(
                    p[:], pghn[:], b_hhn[:], r[:], ALU.add, ALU.mult
                )
                q = smallp.tile([H, C], BF16, tag="q")
                nc.vector.tensor_add(q[:], p[:], pgin[:, cs])
                n = smallp.tile([H, C], BF16, tag="n")
                nc.scalar.activation(n[:], q[:], AF.Tanh, bias=b_ihn[:])
                vb = smallp.tile([H, C], BF16, tag="vb")
                nc.vector.scalar_tensor_tensor(
                    vb[:], z[:], 1.0, n[:], ALU.subtract, ALU.mult
                )
                nc.vector.tensor_sub(hsg[:, cs], zm1[:], vb[:])
                if t < n_steps - 1:
                    mh2 = smallp.tile([H, C], BF16, tag="mh")
                    if s == GS - 1:
                        mbn = refs_next["mb"][:, 0:C]
                    else:
                        mbn = mbt[:, bass.ts(s + 1, C)]
                    nc.vector.tensor_mul(mh2[:], hsg[:, cs], mbn)
                    state["mh"] = mh2
                if pieces is not None:
                    pieces[s]()

        def head_pieces(g, refs):
            """Head MLP for group g, split into GS pieces for interleaving."""
            hsg, xT = refs["hsg"], refs["xT"]
            st = {}

            def piece0():
                p1 = phd.tile([128, COLS], F32, tag="ph")
                nc.tensor.matmul(p1[:], l1h[:], hsg[:], start=True, stop=False,
                                 skip_group_check=True)
                nc.tensor.matmul(p1[:], l1x[:], xT[:], start=False, stop=True,
                                 skip_group_check=True)
                t1 = tmlp.tile([128, COLS], BF16, tag="t1")
                nc.scalar.activation(t1[:], p1[:], AF.Tanh, bias=b1[:])
                st["t1"] = t1

            def piece1():
                p2 = phd.tile([128, COLS], F32, tag="ph")
                nc.tensor.matmul(p2[:], l2[:], st["t1"][:], start=True, stop=True)
                t2 = tmlp.tile([128, COLS], BF16, tag="t2")
                nc.scalar.activation(t2[:], p2[:], AF.Tanh, bias=b2[:])
                st["t2"] = t2

            def piece2():
                p3 = phd.tile([128, COLS], F32, tag="ph")
                nc.tensor.matmul(p3[:AO, :], b3r[:], ones_row[:],
                                 start=True, stop=False, skip_group_check=True)
                nc.tensor.matmul(p3[:AO, :], l3[:], st["t2"][:],
                                 start=False, stop=True, skip_group_check=True)
                o7 = o7p.tile([AO, COLS], F32, tag="o7")
                half = COLS // 2
                nc.vector.tensor_copy(o7[:, 0:half], p3[:AO, 0:half])
                nc.vector.tensor_copy(o7[:, half:COLS], p3[:AO, half:COLS])
                nc.sync.dma_start(out_d[g], o7[:])

            return [piece0, piece1, piece2]

        refs = bulk(0)
        mh0 = smallp.tile([H, C], BF16, tag="mh")
        nc.vector.tensor_mul(mh0[:], h0T[:], refs["mb"][:, 0:C])
        state["mh"] = mh0
        pieces = None
        for g in range(1, ng):
            refs_next = bulk(g)
            chain(g - 1, refs, refs_next, pieces)
            pieces = head_pieces(g - 1, refs)
            refs = refs_next
        chain(ng - 1, refs, None, pieces)
        for pc in head_pieces(ng - 1, refs):
            pc()

    return nc


_BUILT = {}


def get_built(ng):
    if ng not in _BUILT:
        nc = bacc.Bacc(None, target_bir_lowering=False)
        build(nc, ng)
        nc.compile()
        _BUILT[ng] = nc
    return _BUILT[ng]


# --------------------------------------------------------------------------
# host-side packing
# --------------------------------------------------------------------------

def _enumerate_segments(done2):
    """done2 [T,B] -> (seg_env, seg_t0, seg_len) with cuts at done==1.0."""
    starts = done2 == 1.0
    starts[0, :] = True
    nseg_per_env = starts.sum(axis=0)
    seg_env = np.repeat(np.arange(done2.shape[1]), nseg_per_env)
    env_idx, t_idx = np.nonzero(starts.T)
    seg_t0 = t_idx
    # length = next start - this start (within env)
    seg_len = np.empty(len(seg_t0), np.int64)
    pos = 0
    Tn = done2.shape[0]
    for b, k in enumerate(nseg_per_env):
        ts = seg_t0[pos:pos + k]
        seg_len[pos:pos + k - 1] = np.diff(ts)
        seg_len[pos + k - 1] = Tn - ts[-1]
        pos += k
    return seg_env, seg_t0, seg_len


def _pack(done2, h0_nonzero):
    """Bin-pack segments into NLANES lanes.  Returns (N, lane, off) per seg."""
    seg_env, seg_t0, seg_len = _enumerate_segments(done2)
    nseg = len(seg_len)
    total = int(seg_len.sum())
    cap = max(int(seg_len.max()), math.ceil(total / NLANES))
    cap = ((cap + GS - 1) // GS) * GS

    while True:
        rem = np.full(NLANES, cap, np.int64)
        lane = np.full(nseg, -1, np.int64)
        off = np.zeros(nseg, np.int64)
        ok = True
        if h0_nonzero:
            # t=0 segments that continue from h0 must sit at a lane start
            pin = np.nonzero((seg_t0 == 0) & (done2[0, seg_env] != 1.0))[0]
            if len(pin) > NLANES:
                raise RuntimeError("too many h0 segments")
            for j, i in enumerate(pin):
                if seg_len[i] > cap:
                    ok = False
                    break
                lane[i] = j
                off[i] = 0
                rem[j] = cap - seg_len[i]
        if ok:
            order = np.argsort(-seg_len, kind="stable")
            for i in order:
                if lane[i] >= 0:
                    continue
                Lg = seg_len[i]
                j = int(np.argmax(rem >= Lg))
                if rem[j] < Lg:
                    ok = False
                    break
                lane[i] = j
                off[i] = cap - rem[j]
                rem[j] -= Lg
        if ok:
            return cap, seg_env, seg_t0, seg_len, lane, off
        cap += GS


def _prepare(inputs):
    """Host-side pack + permute.  Returns (ng, in_maps, gather_idx)."""
    x = np.ascontiguousarray(np.asarray(inputs["x"], np.float32)).reshape(T, B, OBS)
    done2 = np.ascontiguousarray(
        np.asarray(inputs["done"], np.float32)
    ).reshape(T, B)
    h0 = np.ascontiguousarray(
        np.asarray(inputs["gru_state"], np.float32)
    ).reshape(B, H)
    h0_nonzero = bool(np.any(h0))

    N, seg_env, seg_t0, seg_len, seg_lane, seg_off = _pack(done2, h0_nonzero)
    ng = N // GS

    # flat (src slot) -> (dst slot) index arrays
    reps = seg_len
    src_env = np.repeat(seg_env, reps)
    within = np.concatenate([np.arange(l) for l in seg_len])
    src_t = np.repeat(seg_t0, reps) + within
    dst_lane = np.repeat(seg_lane, reps)
    dst_n = np.repeat(seg_off, reps) + within

    # packed done: copy source done values; padding slots = 1 (reset)
    donep = np.ones((N, NLANES), np.float32)
    donep[dst_n, dst_lane] = done2[src_t, src_env]
    if not h0_nonzero:
        # zero h0 == reset; break any dependence on prior lane garbage
        startmask = within == 0
        donep[dst_n[startmask], dst_lane[startmask]] = 1.0
    mbp_ = (1.0 - donep).astype(BF)                       # [N, NLANES]

    xp = np.zeros((N, NLANES, OBS), BF)
    xp[dst_n, dst_lane] = x[src_t, src_env].astype(BF)

    h0T_all = np.zeros((H, NLANES), BF)
    if h0_nonzero:
        first = within == 0
        fl = dst_lane[first]
        fe = src_env[first]
        fn = dst_n[first]
        sel = fn == 0
        h0T_all[:, fl[sel]] = h0[fe[sel]].T.astype(BF)

    # weights, host-transformed
    w_ih = np.asarray(inputs["w_ih"], np.float32)
    w_hh = np.asarray(inputs["w_hh"], np.float32)
    b_ih = np.asarray(inputs["b_ih"], np.float32)
    b_hh = np.asarray(inputs["b_hh"], np.float32)
    l1 = np.concatenate(
        [np.asarray(inputs["aw1"], np.float32).T,
         np.asarray(inputs["cw1"], np.float32).T], axis=1)       # [128,128]
    l2 = np.zeros((128, 128), np.float32)
    l2[0:64, 0:64] = np.asarray(inputs["aw2"], np.float32).T
    l2[64:128, 64:128] = np.asarray(inputs["cw2"], np.float32).T
    l3 = np.zeros((128, AO), np.float32)
    l3[0:64, 0:A] = np.asarray(inputs["aw3"], np.float32).T
    l3[64:128, A:AO] = np.asarray(inputs["cw3"], np.float32).T
    brz = (b_ih[0:128] + b_hh[0:128]).reshape(128, 1)
    bhhn = b_hh[128:H3].reshape(H, 1)
    bihn = b_ih[128:H3].reshape(H, 1)
    b1 = np.concatenate(
        [np.asarray(inputs["ab1"], np.float32),
         np.asarray(inputs["cb1"], np.float32)]).reshape(128, 1)
    b2 = np.concatenate(
        [np.asarray(inputs["ab2"], np.float32),
         np.asarray(inputs["cb2"], np.float32)]).reshape(128, 1)
    b3 = np.concatenate(
        [np.asarray(inputs["ab3"], np.float32),
         np.asarray(inputs["cb3"], np.float32)]).reshape(AO, 1)

    common = {
        "b3r": np.ascontiguousarray(b3.reshape(1, AO).astype(BF)),
        "w_ihT": np.ascontiguousarray(w_ih.T.astype(BF)),
        "w_hhT": np.ascontiguousarray(w_hh.T.astype(BF)),
        "l1h": np.ascontiguousarray(l1[0:64].astype(BF)),
        "l1x": np.ascontiguousarray(l1[64:128].astype(BF)),
        "l2": l2.astype(BF), "l3": l3.astype(BF),
        "b_rz": brz, "b_hhn": bhhn, "b_ihn": bihn,
        "b1": b1, "b2": b2, "b3": b3,
    }

    in_maps = []
    for c in range(N_CORES):
        sl = slice(c * C, (c + 1) * C)
        # [N, C, OBS] -> [ng, OBS, GS*C] with column order (s, lane)
        xc = xp[:, sl, :].reshape(ng, GS, C, OBS).transpose(0, 3, 1, 2)
        mc = mbp_[:, sl].reshape(ng, GS, C)
        mcb = np.broadcast_to(mc[:, None, :, :], (ng, H, GS, C))
        m = dict(common)
        m["xT"] = np.ascontiguousarray(xc.reshape(ng, OBS, COLS))
        m["mb"] = np.ascontiguousarray(mcb.reshape(ng, H, COLS))
        m["h0T"] = np.ascontiguousarray(h0T_all[:, sl])
        in_maps.append(m)

    gather = (src_t, src_env, dst_n, dst_lane, N, ng)
    return ng, in_maps, gather


def _assemble(per_core_outs, gather):
    src_t, src_env, dst_n, dst_lane, N, ng = gather
    # per-core out [ng, AO, COLS] -> [N, C, AO]
    packed = np.concatenate(
        [
            np.asarray(o, np.float32)
            .reshape(ng, AO, GS, C)
            .transpose(0, 2, 3, 1)
            .reshape(N, C, AO)
            for o in per_core_outs
        ],
        axis=1,
    )                                                     # [N, NLANES, AO]
    full = np.empty((T * B, AO), np.float32)
    full[src_t * B + src_env] = packed[dst_n, dst_lane]
    return full


def run_on_hw(inputs, trace=False, **kw):
    from concourse.bass_utils import run_bass_kernel_spmd

    ng, in_maps, gather = _prepare(inputs)
    nc = get_built(ng)
    res = run_bass_kernel_spmd(
        nc, in_maps, core_ids=list(range(N_CORES)), trace=trace, **kw
    )
    out = _assemble([r["out"] for r in res.results], gather)
    return out, res


def kernel(**inputs):
    out, _ = run_on_hw(inputs)
    return out
